# revision 23
# baseline (speedup 1.0000x reference)
"""MoE (BruteForceMoELinear) Trainium2 kernel — expert-parallel, bf16.

Strategy: one expert per NeuronCore (8 experts / 8 cores).  The host
(inside `kernel()`) dispatches token rows by `gate_idx`, folds the gate
score into the tokens (s >= 0, so relu(W1 (s x)) = s relu(W1 x) and the
whole per-token scale commutes through both GEMMs), pads each expert's
batch to a common capacity C, converts everything to bf16 and hands
core e:

  xt  : (128, KO, C)       = x_e^T (pre-scaled by gate score)
  w1t : (128, KO, d_ff)    = W1_e^T
  w2t : (128, KO, FO, 128) = W2_e^T blocked [f_in, d_blk, f_blk, d_in]

Each core computes  y_e^T = W2_e @ relu(W1_e @ x_e^T)  with bf16
matmuls (full-rate PE) accumulating in f32 PSUM; ReLU is fused into the
PSUM eviction (scalar engine, bf16 out).  The x chunk rides the SP
HWDGE queue while the W1 head block rides the Pool-engine SWDGE queue,
so the two descriptor-generation pipelines overlap and real data hits
the PE ~4.3 us after launch (DMA-complete semaphores cost +900 ns
each); dummy warm-up matmuls keep the PE p-state ramp burned with <100
ns of PE idle before the first real matmul (idle gaps > ~1 us reset the
ramp).  The host scatters per-expert outputs back to token order and
sums the top-k (=2) slots.
"""

import numpy as np

NUM_EXPERT = 8
N_CORES = 8
P = 128
NWARM = 60           # PE p-state warm-up matmuls (64 rows each)
TAILSPLIT = 128       # tokens in the final PSUM group (shortens exit)
TAIL_TRIG = False    # prepared-descriptor exit path (see _build)

_CACHE = {}


def _chunking(maxc):
    """Token capacity C (multiple of 8) and chunk sizes (<=504 each,
    first chunk 256 when possible: 256 bf16 tokens = 512 B contiguous
    DMA runs, the smallest transfer at full DMA efficiency)."""
    c = max(-(-int(maxc) // 8) * 8, 16)
    if c <= 504:
        return [c] if c <= 256 else [256, c - 256]
    chunks = [256]
    rem = c - 256
    n = -(-rem // 504)
    base = -(-rem // (n * 8)) * 8
    while rem > 0:
        t = min(base, rem)
        chunks.append(t)
        rem -= t
    return chunks


def _build(chunks, KO, FO, repeat=1):
    key = (tuple(chunks), KO, FO, repeat)
    if key in _CACHE:
        return _CACHE[key]

    import concourse.mybir as mybir
    import concourse.tile as tile
    from concourse import bacc

    f32 = mybir.dt.float32
    bf16 = mybir.dt.bfloat16
    C = sum(chunks)
    c0 = chunks[0]
    D_FF = FO * P
    NCH = len(chunks)
    offs = np.cumsum([0] + list(chunks))

    nc = bacc.Bacc("TRN2", target_bir_lowering=False, debug=False,
                   num_devices=N_CORES)

    xt = nc.dram_tensor("xt", (P, KO, C), bf16, kind="ExternalInput")
    w1t = nc.dram_tensor("w1t", (P, KO, D_FF), bf16, kind="ExternalInput")
    w2t = nc.dram_tensor("w2t", (P, KO, FO, P), bf16, kind="ExternalInput")
    yt = nc.dram_tensor("yt", (P, KO, C), f32, kind="ExternalOutput")

    # Prepared-descriptor exit path (SWDGE prep + trigger_dma) is kept
    # behind a flag: it saves ~1 us of tail latency in TimelineSim but
    # produced corrupted scatter regions in the full 8-core kernel (the
    # standalone repro in exp_wb.py is numerically exact), so it stays
    # disabled until that divergence is understood.
    use_tail_trig = TAIL_TRIG and chunks[0] == 256
    if use_tail_trig:
        tix = nc.dram_tensor("tix", (P, 8), mybir.dt.int16,
                             kind="ExternalInput")

    NPB = 2 if NCH <= 2 else 1  # PSUM bufs per tag (8 banks total)
    with tile.TileContext(nc) as tc:
        with tc.tile_pool(name="wpool", bufs=1) as wpool, \
             tc.tile_pool(name="ypool", bufs=2) as ypool, \
             tc.tile_pool(name="ps1", bufs=NPB, space="PSUM") as ps1, \
             tc.tile_pool(name="ps2", bufs=NPB, space="PSUM") as ps2:

            bias0 = wpool.tile([P, 1], f32)
            nc.vector.memset(bias0[:], 0.0)
            warm = wpool.tile([P, 64], bf16)
            nc.vector.memset(warm[:], 0.5)

            xsb = wpool.tile([P, KO, C], bf16)
            w1sb = wpool.tile([P, KO, D_FF], bf16)
            w2sb = wpool.tile([P, KO, FO, P], bf16)

            # x chunk0 on the SP/HWDGE queue, W1 head (f 0:256) on the
            # Pool/SWDGE queue: the two descriptor-generation pipelines
            # run in parallel and the transfers queue back-to-back.
            FB = 256
            nc.sync.dma_start(xsb[:, :, 0:c0], xt.ap()[:, :, 0:c0])
            nc.gpsimd.dma_start(w1sb[:, :, 0:FB], w1t.ap()[:, :, 0:FB])

            # PE p-state warm-up: burns the ramp (full clock needs ~3 us
            # from first PE activity) while the first DMAs land.
            wps = ps1.tile([P, c0], f32, name="warm", tag="p1c0")
            for _ in range(NWARM):
                nc.tensor.matmul(wps[:64, :64], warm[:], warm[:],
                                 start=True, stop=True)

            # Rest of the SP stream.  x chunk 1+ goes FIRST: the tile
            # scheduler greedily interleaves both chunks' gemm1s, so the
            # second chunk's tokens must land before the third w1 block
            # is needed or the PE stream stalls on whichever chunk the
            # scheduler picked.
            if NCH > 1:
                nc.sync.dma_start(xsb[:, :, c0:], xt.ap()[:, :, c0:])
            nc.sync.dma_start(w1sb[:, :, FB:2 * FB],
                              w1t.ap()[:, :, FB:2 * FB])
            nc.sync.dma_start(w1sb[:, :, 2 * FB:3 * FB],
                              w1t.ap()[:, :, 2 * FB:3 * FB])
            for fb in range(3, D_FF // FB):
                nc.sync.dma_start(w1sb[:, :, fb * FB:(fb + 1) * FB],
                                  w1t.ap()[:, :, fb * FB:(fb + 1) * FB])
            for do in range(KO):
                nc.sync.dma_start(w2sb[:, do], w2t.ap()[:, do])

            preps = []
            trig_state = {}
            if use_tail_trig:
                c0 = chunks[0]
                ixsb = wpool.tile([P, 8], mybir.dt.int16)
                nc.sync.dma_start(ixsb[:], tix.ap())
                ytail_a = wpool.tile([P, 1, c0 - TAILSPLIT], f32)
                ytail_b = wpool.tile([P, 1, TAILSPLIT], f32)
                dsem = nc.alloc_semaphore("tail_dma")
                preps.append(nc.gpsimd.dma_scatter_add(
                    yt.ap()[:, KO - 1, 0:c0 - TAILSPLIT],
                    ytail_a[:, :, :], ixsb[:], P, P, c0 - TAILSPLIT,
                    elem_step=KO * C, prepare_only=True, sem=dsem,
                    queue_num=0))
                preps.append(nc.gpsimd.dma_scatter_add(
                    yt.ap()[:, KO - 1, c0 - TAILSPLIT:c0],
                    ytail_b[:, :, :], ixsb[:], P, P, TAILSPLIT,
                    elem_step=KO * C, prepare_only=True, sem=dsem,
                    queue_num=0))
                trig_state.update(dsem=dsem, ytail_a=ytail_a,
                                  ytail_b=ytail_b)

            hs = [wpool.tile([P, FO, chunks[ch]], bf16, name=f"h{ch}")
                  for ch in range(NCH)]

            def gemm1(ch, fo):
                p1 = ps1.tile([P, chunks[ch]], f32, name="p1",
                              tag=f"p1c{ch}")
                for ko in range(KO):
                    nc.tensor.matmul(p1[:], w1sb[:, ko, fo * P:(fo + 1) * P],
                                     xsb[:, ko, offs[ch]:offs[ch + 1]],
                                     start=(ko == 0), stop=(ko == KO - 1))
                nc.scalar.activation(hs[ch][:, fo, :], p1[:],
                                     mybir.ActivationFunctionType.Relu,
                                     bias=bias0[:])

            def gemm2_tail(ch, do, lo, hi, ystage):
                n = hi - lo
                p2 = ps2.tile([P, chunks[ch]], f32, name="p2",
                              tag=f"p2c{ch}")
                for fo in range(FO):
                    nc.tensor.matmul(p2[:, 0:n], w2sb[:, do, fo, :],
                                     hs[ch][:, fo, lo:hi],
                                     start=(fo == 0), stop=(fo == FO - 1))
                nc.scalar.copy(ystage[:, 0, :], p2[:, 0:n])

            def gemm2(ch, do, lo=0, hi=None, evict="act"):
                hi = chunks[ch] if hi is None else hi
                n = hi - lo
                p2 = ps2.tile([P, chunks[ch]], f32, name="p2",
                              tag=f"p2c{ch}")
                for fo in range(FO):
                    nc.tensor.matmul(p2[:, 0:n], w2sb[:, do, fo, :],
                                     hs[ch][:, fo, lo:hi],
                                     start=(fo == 0), stop=(fo == FO - 1))
                ysb = ypool.tile([P, chunks[ch]], f32, tag=f"y{ch}",
                                 name="ysb")
                if evict == "act":
                    nc.scalar.copy(ysb[:, 0:n], p2[:, 0:n])
                    nc.sync.dma_start(
                        yt.ap()[:, do, offs[ch] + lo:offs[ch] + hi],
                        ysb[:, 0:n])
                else:
                    # tail overlap: eviction on DVE and the DMA on the
                    # DVE queue, in parallel with the Act/SP chains of
                    # the neighbouring groups
                    nc.vector.tensor_scalar_mul(ysb[:, 0:n], p2[:, 0:n],
                                                1.0)
                    nc.vector.dma_start(
                        yt.ap()[:, do, offs[ch] + lo:offs[ch] + hi],
                        ysb[:, 0:n])

            for _ in range(repeat):
                # Phase 1: h = relu(W1 x).  Head f-blocks chunk-major so
                # chunk 1+'s x DMA can land; tail f-blocks fo-major.
                HEAD = 6 if NCH > 1 else FO
                for ch in range(NCH):
                    for fo in range(min(HEAD, FO)):
                        gemm1(ch, fo)
                for fo in range(HEAD, FO):
                    for ch in range(NCH):
                        gemm1(ch, fo)
                # Phase 2: y = W2 h, d-block-major.  The last do block
                # ends on chunk 0, split so the final PSUM group (and
                # its eviction + exit DMA) covers only TAILSPLIT tokens;
                # both split groups leave via the prepared-descriptor
                # trigger path when available.
                c0 = chunks[0]
                for do in range(KO):
                    if do < KO - 1:
                        for ch in range(NCH):
                            gemm2(ch, do)
                    else:
                        for ch in range(1, NCH):
                            gemm2(ch, do)
                        if use_tail_trig:
                            gemm2_tail(0, do, 0, c0 - TAILSPLIT,
                                       trig_state["ytail_a"])
                            gemm2_tail(0, do, c0 - TAILSPLIT, c0,
                                       trig_state["ytail_b"])
                            nc.gpsimd.trigger_dma(count=None, queue_num=0)
                            nc.sync.wait_ge(trig_state["dsem"], 16 * 2)
                        elif c0 > TAILSPLIT:
                            gemm2(0, do, 0, c0 - TAILSPLIT)
                            gemm2(0, do, c0 - TAILSPLIT, c0)
                        else:
                            gemm2(0, do)

    if preps:
        # The tile wait-pass accounts each SWDGE prep on a DMASW lane,
        # but nothing increments lane sems on the prepared path (the
        # trigger fires the DMA; its completion bumps only the sem baked
        # into the descriptors).  Reduce the epilogue lane waits by 16
        # per prep; data completion is barriered by the explicit dsem
        # wait emitted above.
        from collections import Counter
        lane_of = {11 + k: f"DMASW{k}" for k in range(8)}
        prep_lanes = Counter(lane_of.get(p.ins.bass_scheduled_proc)
                             for p in preps)
        for blk in nc.m.functions[0].blocks:
            for inst in blk.instructions:
                si = inst.sync_info
                if si is None:
                    continue
                for w in list(si.on_wait):
                    nm = str(w.ant_name).split("_")[0]
                    if nm in prep_lanes and w.wait_value is not None:
                        w.wait_value = max(
                            0, w.wait_value - 16 * prep_lanes[nm])

    nc.compile()
    _CACHE[key] = nc
    return nc


_last = {}


def kernel(inp, gate_idx, gate_score, w_htoh4, w_h4toh):
    import ml_dtypes

    bf16 = ml_dtypes.bfloat16
    inp = np.asarray(inp, dtype=np.float32)
    gate_idx = np.asarray(gate_idx)
    gate_score = np.asarray(gate_score, dtype=np.float32)
    w_htoh4 = np.asarray(w_htoh4, dtype=np.float32)
    w_h4toh = np.asarray(w_h4toh, dtype=np.float32)

    B, d_model = inp.shape
    n_expert, d_ff, _ = w_htoh4.shape
    assert n_expert == NUM_EXPERT
    KO = d_model // P
    FO = d_ff // P

    gi = gate_idx.astype(np.int64)
    order = np.argsort(gi, kind="stable")
    counts = np.bincount(gi, minlength=NUM_EXPERT)
    idx_split = np.split(order, np.cumsum(counts)[:-1])

    chunks = _chunking(counts.max())
    C = sum(chunks)

    # fold per-row gate score into x (scores >= 0 commute with relu)
    scores_flat = gate_score.reshape(-1)
    xs = inp * scores_flat[:, None]

    nc = _build(chunks, KO, FO)

    # identity scatter indices, wrapped over 16 partitions (token i's
    # index lives at [i % 16, i // 16]; value = destination row i)
    tix_h = np.zeros((P, 8), dtype=np.int16)
    for i in range(P):
        tix_h[i % 16, i // 16] = i

    in_maps = []
    for e in range(NUM_EXPERT):
        idx = idx_split[e]
        cnt = len(idx)
        xT = np.zeros((d_model, C), dtype=np.float32)
        if cnt:
            xT[:, :cnt] = xs[idx].T
        xt_h = np.ascontiguousarray(
            xT.reshape(KO, P, C).transpose(1, 0, 2)).astype(bf16)
        w1_h = np.ascontiguousarray(
            w_htoh4[e].T.reshape(KO, P, d_ff).transpose(1, 0, 2)).astype(bf16)
        # W2^T blocked: w2t[p, do, fo, d] = W2[do*128+d, fo*128+p]
        w2_h = np.ascontiguousarray(
            w_h4toh[e].T.reshape(FO, P, KO, P).transpose(1, 2, 0, 3)
        ).astype(bf16)
        m = {"xt": xt_h, "w1t": w1_h, "w2t": w2_h}
        if TAIL_TRIG and chunks[0] == 256:
            m["tix"] = tix_h
        in_maps.append(m)

    from concourse import bass_utils
    res = bass_utils.run_bass_kernel_spmd(nc, in_maps,
                                          core_ids=list(range(N_CORES)))

    _last.update(nc=nc, in_maps=in_maps, res=res, chunks=chunks,
                 KO=KO, FO=FO)

    y_full = np.empty((B, d_model), dtype=np.float32)
    for e in range(NUM_EXPERT):
        idx = idx_split[e]
        if len(idx) == 0:
            continue
        yt_h = np.asarray(res.results[e]["yt"], dtype=np.float32)
        yT = yt_h.transpose(1, 0, 2).reshape(d_model, C)
        y_full[idx] = yT[:, :len(idx)].T

    out = y_full[0::2] + y_full[1::2]
    return np.ascontiguousarray(out, dtype=np.float32)
